# revision 25
# baseline (speedup 1.0000x reference)
"""MemristorDense forward on 8 Trainium2 NeuronCores.

Math
----
Reference computes, with R = n_in+1 rows (x plus a ones bias-row), C = 2*n_out
interleaved pos/neg columns:

    y = 0.5 * sum_r sign(x) * (W+m9) * exp(L[b,r] * log2(n[r,c]))

with L = ln(max(2|x|,1e-12)), m9 = max_w/9.  Write n = 2^gbar * (1+v)
(gbar = midrange of log2(n), |v| <~ 0.26) and z = log2(2|x|).  Then
exp(L*log2 n) = (2|x|)^gbar * (1+v)^z, and the binomial series
(1+v)^z = sum_k C(z,k) v^k turns the [B,R,C] elementwise-pow contraction
into K+1 TensorEngine matmuls.  K=2 suffices (~2e-3 relative against the
2e-2 gate; the fp32 reference itself is ~3e-5 from a float64 oracle).

Because the pos and neg columns of an output pair share the same A-side,
their weight products are pre-added on the host (exact, in float64):

    y[:,j] = sum_k A_k.T @ Wt_k[:,j],  Wt_k = (wp+m9)*vp^k - (wn+m9)*vn^k
    A_0 = x*|x|^(gbar-1) * 2^(gbar-1),  A_1 = A_0*z,  A_2 = A_1*(z-1)/2

so the device runs no W-side elementwise work at all, the matmul free dim
drops to 64, and no pos-neg fold is needed after PSUM.  The ones bias-row is
also removed from the series: its contribution (b+m9)*n[n_in,c] (pos-neg
combined) is b-independent and exact, folded into a k=0-only contraction
chunk whose A-column is 0.5.

Everything on device is fp16; accumulation is fp32 in PSUM.  ScalarE runs
only Ln and Exp (one act-table set, load hoisted before data arrives);
x^2 and the A-chain run on VectorE.  DMAs are column-split across the two
DMA streams (sync HWDGE + gpsimd SWDGE) aligned with the compute slices,
and staged tile_wait_until hints order the engine FIFOs (the scheduler's
DMA model is optimistic and would otherwise head-of-line-block the A path).

Sharding: tensor-parallel over output columns (64 per core), A-side
replicated -- no collectives, gather is a pure concat.

Device layout: x tiles are [128, 8*128] with tile[p, 128*ch + b] =
x[b, 128*ch + p]; Wt tiles are [128, 64-col chunks] with
tile[p, 64*ch + c] = Wt[128*ch + p, c].
"""

import numpy as np

import concourse.bacc as bacc
import concourse.tile as tile
import concourse.mybir as mybir
from concourse.bass_utils import run_bass_kernel_spmd

F32 = mybir.dt.float32
F16 = mybir.dt.float16
ALU = mybir.AluOpType
ACT = mybir.ActivationFunctionType

NCORES = 8
B = 128
N_IN = 1024
N_OUT = 512
NCH = 8                 # full 128-row chunks of real x rows
RC = NCH * 128          # 1024 real contraction rows
RP = RC + 128           # + bias chunk columns on the A side (k=0 only)
CS = N_OUT // NCORES    # 64 output columns per core
LN2 = 0.6931471805599453
# wc_in column offsets: [bias | Wt0 chunks | Wt1 chunks | Wt2 chunks]
WT0 = 64
WT1 = WT0 + NCH * CS
WT2 = WT1 + NCH * CS
WCOLS = WT2 + NCH * CS

# Stashed by kernel() for the test harness (exec_time_ns, trace paths).
LAST_RESULTS = None

_ACT_SET = "natural_log_exp_and_others"
_ACT_SHARED = {
    ACT.Square, ACT.Ln, ACT.Exp, ACT.Copy, ACT.Identity, ACT.Abs, ACT.Sign,
    ACT.MemsetZero,
}


def _patched_tables(arch, _orig=bacc.get_activation_tables):
    """Steer the act-table-load pass to a single table set: every function we
    use (ln/exp/copy) lives in natural_log_exp_and_others, but the greedy
    per-instruction chooser would otherwise pick several sets (~1.3us
    ACT_TABLE_LOAD each on the critical ScalarE chain).  Set names and order
    are preserved so act_func_set_id stays a valid act_info.json index."""
    t = _orig(arch)
    return {
        name: (funcs if name == _ACT_SET else (funcs - _ACT_SHARED))
        for name, funcs in t.items()
    }


def _build_program(gbar: float):
    orig_tables = bacc.get_activation_tables
    bacc.get_activation_tables = _patched_tables
    try:
        return _build_program_inner(gbar)
    finally:
        bacc.get_activation_tables = orig_tables


def _build_program_inner(gbar: float):
    nc = bacc.Bacc(
        "TRN2", target_bir_lowering=False, debug=False, num_devices=NCORES
    )
    xt_d = nc.dram_tensor("xt_in", [128, RC], F16, kind="ExternalInput").ap()
    wc_d = nc.dram_tensor("wc_in", [128, WCOLS], F16, kind="ExternalInput").ap()
    y_d = nc.dram_tensor("y_out", [B, CS], F32, kind="ExternalOutput").ap()

    with tile.TileContext(nc) as tc:
        with (
            tc.tile_pool(name="pers", bufs=1) as pool,
            tc.tile_pool(name="acc", bufs=1, space="PSUM") as pspool,
        ):
            eps = pool.tile([128, 1], F32)
            nc.vector.memset(eps[:], 1e-24)
            xT = pool.tile([128, RC], F16)
            Ax = pool.tile([128, RC], F16)
            Lr = pool.tile([128, RC], F16)
            E1 = pool.tile([128, RC], F16)
            Z = pool.tile([128, RC], F16)
            Z1h = pool.tile([128, RC], F16)
            A0 = pool.tile([128, RP], F16)
            A1 = pool.tile([128, RC], F16)
            A2 = pool.tile([128, RC], F16)
            Wc = pool.tile([128, WCOLS], F16)
            ysb = pool.tile([128, CS], F32)
            acc = pspool.tile([128, CS], F32)

            # bias chunk of A0: 0.5 on partition 0, zero elsewhere
            nc.vector.memset(A0[:, RC:RP], 0.0)
            nc.vector.memset(A0[0:1, RC:RP], 0.5)

            # Column halves aligned with the DMA split and chunk groups.
            H = 512
            sl_a = slice(0, H)
            sl_b = slice(H, RC)

            # Input DMA on two parallel streams; x pieces first (they head
            # the serial ScalarE chain), weight products behind them.
            # ScalarE issues no input DMA: its queue starts with the hoisted
            # act-table load.
            nc.sync.dma_start(xT[:, sl_a], xt_d[:, sl_a])
            nc.gpsimd.dma_start(xT[:, sl_b], xt_d[:, sl_b])
            nc.sync.dma_start(Wc[:, 0:WT2], wc_d[:, 0:WT2])
            nc.gpsimd.dma_start(Wc[:, WT2:WCOLS], wc_d[:, WT2:WCOLS])

            # x chain per half: Ax = x^2 on DVE; Lr = ln(4x^2+eps) =
            # 2 ln(2|x|) on ScalarE (4x folded into the Ln input scale);
            # E1 = (2|x|)^(gbar-1) on ScalarE (same act-table set as Ln).
            # DVE: z = Lr/(2 ln2) ; (z-1)/2 = Lr/(4 ln2) - 1/2 ;
            # A0 = x*E1 ; A1 = A0*z ; A2 = A1*(z-1)/2.
            def x_chain(sl):
                nc.vector.tensor_mul(Ax[:, sl], xT[:, sl], xT[:, sl])
                nc.scalar.activation(
                    Lr[:, sl], Ax[:, sl], ACT.Ln, bias=eps[:], scale=4.0
                )
                nc.scalar.activation(
                    E1[:, sl], Lr[:, sl], ACT.Exp, scale=(gbar - 1.0) / 2.0
                )
                nc.vector.tensor_scalar(
                    Z[:, sl], Lr[:, sl], 1.0 / (2 * LN2), None, ALU.mult
                )
                nc.vector.tensor_scalar(
                    Z1h[:, sl], Lr[:, sl], 1.0 / (4 * LN2), -0.5,
                    ALU.mult, ALU.add,
                )
                nc.vector.tensor_mul(A0[:, sl], xT[:, sl], E1[:, sl])
                nc.vector.tensor_mul(A1[:, sl], A0[:, sl], Z[:, sl])
                nc.vector.tensor_mul(A2[:, sl], A1[:, sl], Z1h[:, sl])

            x_chain(sl_a)
            with tc.tile_wait_until(0.0045):
                x_chain(sl_b)

            # 25 matmuls, one PSUM accumulation group, N=64 each.  PE executes
            # its FIFO in order, so group [bias, k0a, k1a, k0b, k1b, k2a, k2b]:
            # a-half matmuls stream while the b-half chain still computes.
            nc.tensor.matmul(
                acc[:], A0[:, RC:RP], Wc[:, 0:WT0], start=True, stop=False
            )
            CHA = H // 128
            groups = [
                (A0, WT0, 0, CHA), (A1, WT1, 0, CHA),
                (A0, WT0, CHA, NCH), (A1, WT1, CHA, NCH),
                (A2, WT2, 0, CHA), (A2, WT2, CHA, NCH),
            ]
            for gi, (Ak, off, c0, c1) in enumerate(groups):
                for ch in range(c0, c1):
                    wsl = slice(off + ch * CS, off + (ch + 1) * CS)
                    asl = slice(ch * 128, (ch + 1) * 128)
                    nc.tensor.matmul(
                        acc[:], Ak[:, asl], Wc[:, wsl],
                        start=False,
                        stop=(gi == len(groups) - 1 and ch == c1 - 1),
                    )

            # pos/neg were pre-added on the host: PSUM already holds y
            nc.scalar.copy(ysb[:], acc[:])
            nc.sync.dma_start(y_d[0:64, :], ysb[0:64, :])
            nc.scalar.dma_start(y_d[64:128, :], ysb[64:128, :])

    nc.compile()
    return nc


def _shard_inputs(x, w_pos, w_neg, b_pos, b_neg, n_param, m9, gbar):
    """Per-core input maps: slicing, layout swizzles, dtype casts, and weight
    preprocessing: Wt_k = (wp+m9)*vp^k - (wn+m9)*vn^k with v = n*2^-gbar - 1,
    computed in float64 and rounded to fp16 once; exact bias-row fold."""

    def swizzle(host, w):  # [nch*128, w] -> [128, nch*w] device layout
        nch = host.shape[0] // 128
        return np.ascontiguousarray(
            host.reshape(nch, 128, w).transpose(1, 0, 2).reshape(128, nch * w)
        )

    # xT[p, 128*ch + b] = x[b, 128*ch + p]
    xT = np.ascontiguousarray(
        x.astype(np.float16).reshape(128, NCH, 128).transpose(2, 1, 0).reshape(128, RC)
    )

    W0p = w_pos.astype(np.float64) + m9
    W0n = -(w_neg.astype(np.float64) + m9)
    sv = np.float64(2.0) ** (-gbar)
    vp = n_param[:N_IN, 0::2].astype(np.float64) * sv - 1.0
    vn = n_param[:N_IN, 1::2].astype(np.float64) * sv - 1.0
    Wt0 = W0p + W0n
    Wt1 = W0p * vp + W0n * vn
    Wt2 = W0p * vp * vp + W0n * vn * vn
    # exact bias-row fold: contribution (b+m9)*n[N_IN, c], pos-neg combined
    nb = n_param[N_IN, :].astype(np.float64)
    yb = (b_pos.astype(np.float64) + m9) * nb[0::2] - (
        b_neg.astype(np.float64) + m9
    ) * nb[1::2]

    in_maps = []
    for j in range(NCORES):
        cp = slice(CS * j, CS * (j + 1))
        # device A-column is 0.5, so the row carries yb and contributes yb/2
        bias_host = np.zeros((128, CS), np.float16)
        bias_host[0, :] = yb[cp]
        wc = np.concatenate(
            [
                swizzle(bias_host, CS),
                swizzle(Wt0[:, cp].astype(np.float16), CS),
                swizzle(Wt1[:, cp].astype(np.float16), CS),
                swizzle(Wt2[:, cp].astype(np.float16), CS),
            ],
            axis=1,
        )
        in_maps.append(
            {
                "xt_in": xT,
                "wc_in": np.ascontiguousarray(wc),
            }
        )
    return in_maps


def kernel(x, w_pos, w_neg, b_pos, b_neg, n_param, **run_kwargs):
    global LAST_RESULTS
    x = np.ascontiguousarray(np.asarray(x, np.float32))
    w_pos = np.asarray(w_pos, np.float32)
    w_neg = np.asarray(w_neg, np.float32)
    b_pos = np.asarray(b_pos, np.float32)
    b_neg = np.asarray(b_neg, np.float32)
    n_param = np.asarray(n_param, np.float32)

    max_w = float(max(w_pos.max(), w_neg.max(), b_pos.max(), b_neg.max()))
    m9 = max_w / 9.0
    gbar = float(
        0.5 * (np.log2(float(n_param.min())) + np.log2(float(n_param.max())))
    )

    nc = _build_program(gbar)
    in_maps = _shard_inputs(x, w_pos, w_neg, b_pos, b_neg, n_param, m9, gbar)
    res = run_bass_kernel_spmd(nc, in_maps, list(range(NCORES)), **run_kwargs)
    LAST_RESULTS = res
    return np.concatenate([res.results[j]["y_out"] for j in range(NCORES)], axis=1)


# revision 27
# speedup vs baseline: 1.1118x; 1.1118x over previous
"""MemristorDense forward on 8 Trainium2 NeuronCores.

Math
----
Reference computes, with R = n_in+1 rows (x plus a ones bias-row), C = 2*n_out
interleaved pos/neg columns:

    y = 0.5 * sum_r sign(x) * (W+m9) * exp(L[b,r] * log2(n[r,c]))

with L = ln(max(2|x|,1e-12)), m9 = max_w/9.  Write n = 2^gbar * (1+v)
(gbar = midrange of log2(n), |v| <~ 0.26) and z = log2(2|x|).  Then
exp(L*log2 n) = (2|x|)^gbar * (1+v)^z, and the binomial series
(1+v)^z = sum_k C(z,k) v^k turns the [B,R,C] elementwise-pow contraction
into K+1 TensorEngine matmuls.  K=2 suffices (~2e-3 relative against the
2e-2 gate; the fp32 reference itself is ~3e-5 from a float64 oracle).

Because the pos and neg columns of an output pair share the same A-side,
their weight products are pre-added on the host (exact, in float64):

    y[:,j] = sum_k A_k.T @ Wt_k[:,j],  Wt_k = (wp+m9)*vp^k - (wn+m9)*vn^k
    A_0 = x*|x|^(gbar-1) * 2^(gbar-1),  A_1 = A_0*z,  A_2 = A_1*(z-1)/2

so the device runs no W-side elementwise work at all, the matmul free dim
drops to 64, and no pos-neg fold is needed after PSUM.  The ones bias-row is
also removed from the series: its contribution (b+m9)*n[n_in,c] (pos-neg
combined) is b-independent and exact, folded into a k=0-only contraction
chunk whose A-column is 0.5.

Everything on device is fp16; accumulation is fp32 in PSUM.  ScalarE runs
only Ln and Exp (one act-table set, load hoisted before data arrives);
x^2 and the A-chain run on VectorE.  DMAs are column-split across the two
DMA streams (sync HWDGE + gpsimd SWDGE) aligned with the compute slices,
and staged tile_wait_until hints order the engine FIFOs (the scheduler's
DMA model is optimistic and would otherwise head-of-line-block the A path).

Sharding: tensor-parallel over output columns (64 per core), A-side
replicated -- no collectives, gather is a pure concat.

Device layout: x tiles are [128, 8*128] with tile[p, 128*ch + b] =
x[b, 128*ch + p]; Wt tiles are [128, 64-col chunks] with
tile[p, 64*ch + c] = Wt[128*ch + p, c].
"""

import numpy as np

import concourse.bacc as bacc
import concourse.tile as tile
import concourse.mybir as mybir
from concourse.bass_utils import run_bass_kernel_spmd

F32 = mybir.dt.float32
F16 = mybir.dt.float16
ALU = mybir.AluOpType
ACT = mybir.ActivationFunctionType

NCORES = 8
B = 128
N_IN = 1024
N_OUT = 512
NCH = 8                 # full 128-row chunks of real x rows
RC = NCH * 128          # 1024 real contraction rows
RP = RC + 128           # + bias chunk columns on the A side (k=0 only)
CS = N_OUT // NCORES    # 64 output columns per core
LN2 = 0.6931471805599453
# wc_in column offsets: [bias | Wt0 chunks | Wt1 chunks | Wt2 chunks]
WT0 = 64
WT1 = WT0 + NCH * CS
WT2 = WT1 + NCH * CS
WCOLS = WT2 + NCH * CS

# Stashed by kernel() for the test harness (exec_time_ns, trace paths).
LAST_RESULTS = None

_ACT_SET = "natural_log_exp_and_others"
_ACT_SHARED = {
    ACT.Square, ACT.Ln, ACT.Exp, ACT.Copy, ACT.Identity, ACT.Abs, ACT.Sign,
    ACT.MemsetZero,
}


def _patched_tables(arch, _orig=bacc.get_activation_tables):
    """Steer the act-table-load pass to a single table set: every function we
    use (ln/exp/copy) lives in natural_log_exp_and_others, but the greedy
    per-instruction chooser would otherwise pick several sets (~1.3us
    ACT_TABLE_LOAD each on the critical ScalarE chain).  Set names and order
    are preserved so act_func_set_id stays a valid act_info.json index."""
    t = _orig(arch)
    return {
        name: (funcs if name == _ACT_SET else (funcs - _ACT_SHARED))
        for name, funcs in t.items()
    }


def _build_program(gbar: float):
    orig_tables = bacc.get_activation_tables
    bacc.get_activation_tables = _patched_tables
    try:
        return _build_program_inner(gbar)
    finally:
        bacc.get_activation_tables = orig_tables


def _build_program_inner(gbar: float):
    nc = bacc.Bacc(
        "TRN2", target_bir_lowering=False, debug=False, num_devices=NCORES
    )
    xt_d = nc.dram_tensor("xt_in", [128, RC], F16, kind="ExternalInput").ap()
    wc_d = nc.dram_tensor("wc_in", [128, WCOLS], F16, kind="ExternalInput").ap()
    y_d = nc.dram_tensor("y_out", [B, CS], F32, kind="ExternalOutput").ap()

    with tile.TileContext(nc) as tc:
        with (
            tc.tile_pool(name="pers", bufs=1) as pool,
            tc.tile_pool(name="acc", bufs=1, space="PSUM") as pspool,
        ):
            eps = pool.tile([128, 1], F32)
            nc.vector.memset(eps[:], 1e-24)
            xT = pool.tile([128, RC], F16)
            Ax = pool.tile([128, RC], F16)
            Lr = pool.tile([128, RC], F16)
            E1 = pool.tile([128, RC], F16)
            Z = pool.tile([128, RC], F16)
            Z1h = pool.tile([128, RC], F16)
            A0 = pool.tile([128, RP], F16)
            A1 = pool.tile([128, RC], F16)
            A2 = pool.tile([128, RC], F16)
            Wc = pool.tile([128, WCOLS], F16)
            ysb = pool.tile([128, CS], F32)
            acc = pspool.tile([128, CS], F32)

            # bias chunk of A0: 0.5 on partition 0, zero elsewhere
            nc.vector.memset(A0[:, RC:RP], 0.0)
            nc.vector.memset(A0[0:1, RC:RP], 0.5)

            # Column halves aligned with the DMA split and chunk groups.
            H = 512
            sl_a = slice(0, H)
            sl_b = slice(H, RC)

            # Input DMA spread over the three issue queues; x-a pieces first
            # (they head the serial ScalarE chain; partition-split over the
            # two fast HWDGE queues), then the weight products in k-order to
            # match their matmul deadlines.  ScalarE's x DMA issue precedes
            # its act-table load; everything else ScalarE does comes after.
            nc.sync.dma_start(xT[0:64, sl_a], xt_d[0:64, sl_a])
            nc.scalar.dma_start(xT[64:128, sl_a], xt_d[64:128, sl_a])
            nc.gpsimd.dma_start(xT[:, sl_b], xt_d[:, sl_b])
            nc.sync.dma_start(Wc[:, 0:WT1], wc_d[:, 0:WT1])
            nc.sync.dma_start(Wc[:, WT1:WT2], wc_d[:, WT1:WT2])
            nc.gpsimd.dma_start(Wc[:, WT2:WCOLS], wc_d[:, WT2:WCOLS])

            # x chain per half: Ax = x^2 on DVE; Lr = ln(4x^2+eps) =
            # 2 ln(2|x|) on ScalarE (4x folded into the Ln input scale);
            # E1 = (2|x|)^(gbar-1) on ScalarE (same act-table set as Ln).
            # DVE: z = Lr/(2 ln2) ; (z-1)/2 = Lr/(4 ln2) - 1/2 ;
            # A0 = x*E1 ; A1 = A0*z ; A2 = A1*(z-1)/2.
            def x_chain(sl):
                nc.vector.tensor_mul(Ax[:, sl], xT[:, sl], xT[:, sl])
                nc.scalar.activation(
                    Lr[:, sl], Ax[:, sl], ACT.Ln, bias=eps[:], scale=4.0
                )
                nc.scalar.activation(
                    E1[:, sl], Lr[:, sl], ACT.Exp, scale=(gbar - 1.0) / 2.0
                )
                nc.vector.tensor_scalar(
                    Z[:, sl], Lr[:, sl], 1.0 / (2 * LN2), None, ALU.mult
                )
                nc.vector.tensor_scalar(
                    Z1h[:, sl], Lr[:, sl], 1.0 / (4 * LN2), -0.5,
                    ALU.mult, ALU.add,
                )
                nc.vector.tensor_mul(A0[:, sl], xT[:, sl], E1[:, sl])
                nc.vector.tensor_mul(A1[:, sl], A0[:, sl], Z[:, sl])
                nc.vector.tensor_mul(A2[:, sl], A1[:, sl], Z1h[:, sl])

            x_chain(sl_a)
            with tc.tile_wait_until(0.0045):
                x_chain(sl_b)

            # 25 matmuls, one PSUM accumulation group, N=64 each.  PE executes
            # its FIFO in order, so group [bias, k0a, k1a, k0b, k1b, k2a, k2b]:
            # a-half matmuls stream while the b-half chain still computes.
            nc.tensor.matmul(
                acc[:], A0[:, RC:RP], Wc[:, 0:WT0], start=True, stop=False
            )
            CHA = H // 128
            groups = [
                (A0, WT0, 0, CHA), (A1, WT1, 0, CHA), (A2, WT2, 0, CHA),
                (A0, WT0, CHA, NCH), (A1, WT1, CHA, NCH), (A2, WT2, CHA, NCH),
            ]
            for gi, (Ak, off, c0, c1) in enumerate(groups):
                for ch in range(c0, c1):
                    wsl = slice(off + ch * CS, off + (ch + 1) * CS)
                    asl = slice(ch * 128, (ch + 1) * 128)
                    nc.tensor.matmul(
                        acc[:], Ak[:, asl], Wc[:, wsl],
                        start=False,
                        stop=(gi == len(groups) - 1 and ch == c1 - 1),
                    )

            # pos/neg were pre-added on the host: PSUM already holds y
            nc.scalar.copy(ysb[:], acc[:])
            nc.sync.dma_start(y_d[0:64, :], ysb[0:64, :])
            nc.scalar.dma_start(y_d[64:128, :], ysb[64:128, :])

    nc.compile()
    return nc


def _shard_inputs(x, w_pos, w_neg, b_pos, b_neg, n_param, m9, gbar):
    """Per-core input maps: slicing, layout swizzles, dtype casts, and weight
    preprocessing: Wt_k = (wp+m9)*vp^k - (wn+m9)*vn^k with v = n*2^-gbar - 1,
    computed in float64 and rounded to fp16 once; exact bias-row fold."""

    def swizzle(host, w):  # [nch*128, w] -> [128, nch*w] device layout
        nch = host.shape[0] // 128
        return np.ascontiguousarray(
            host.reshape(nch, 128, w).transpose(1, 0, 2).reshape(128, nch * w)
        )

    # xT[p, 128*ch + b] = x[b, 128*ch + p]
    xT = np.ascontiguousarray(
        x.astype(np.float16).reshape(128, NCH, 128).transpose(2, 1, 0).reshape(128, RC)
    )

    W0p = w_pos.astype(np.float64) + m9
    W0n = -(w_neg.astype(np.float64) + m9)
    sv = np.float64(2.0) ** (-gbar)
    vp = n_param[:N_IN, 0::2].astype(np.float64) * sv - 1.0
    vn = n_param[:N_IN, 1::2].astype(np.float64) * sv - 1.0
    Wt0 = W0p + W0n
    Wt1 = W0p * vp + W0n * vn
    Wt2 = W0p * vp * vp + W0n * vn * vn
    # exact bias-row fold: contribution (b+m9)*n[N_IN, c], pos-neg combined
    nb = n_param[N_IN, :].astype(np.float64)
    yb = (b_pos.astype(np.float64) + m9) * nb[0::2] - (
        b_neg.astype(np.float64) + m9
    ) * nb[1::2]

    in_maps = []
    for j in range(NCORES):
        cp = slice(CS * j, CS * (j + 1))
        # device A-column is 0.5, so the row carries yb and contributes yb/2
        bias_host = np.zeros((128, CS), np.float16)
        bias_host[0, :] = yb[cp]
        wc = np.concatenate(
            [
                swizzle(bias_host, CS),
                swizzle(Wt0[:, cp].astype(np.float16), CS),
                swizzle(Wt1[:, cp].astype(np.float16), CS),
                swizzle(Wt2[:, cp].astype(np.float16), CS),
            ],
            axis=1,
        )
        in_maps.append(
            {
                "xt_in": xT,
                "wc_in": np.ascontiguousarray(wc),
            }
        )
    return in_maps


def kernel(x, w_pos, w_neg, b_pos, b_neg, n_param, **run_kwargs):
    global LAST_RESULTS
    x = np.ascontiguousarray(np.asarray(x, np.float32))
    w_pos = np.asarray(w_pos, np.float32)
    w_neg = np.asarray(w_neg, np.float32)
    b_pos = np.asarray(b_pos, np.float32)
    b_neg = np.asarray(b_neg, np.float32)
    n_param = np.asarray(n_param, np.float32)

    max_w = float(max(w_pos.max(), w_neg.max(), b_pos.max(), b_neg.max()))
    m9 = max_w / 9.0
    gbar = float(
        0.5 * (np.log2(float(n_param.min())) + np.log2(float(n_param.max())))
    )

    nc = _build_program(gbar)
    in_maps = _shard_inputs(x, w_pos, w_neg, b_pos, b_neg, n_param, m9, gbar)
    res = run_bass_kernel_spmd(nc, in_maps, list(range(NCORES)), **run_kwargs)
    LAST_RESULTS = res
    return np.concatenate([res.results[j]["y_out"] for j in range(NCORES)], axis=1)
